# revision 11
# baseline (speedup 1.0000x reference)
"""Trainium2 Bass kernel for nn_BusinessCostLoss (weighted binary CE loss).

Reference math (per task, per element, labels y in {0,1}):
    d    = l1 - l0
    base = -log(softmax(l)[y]) = log(1 + exp(-(2y-1)*d))  (eps=1e-8 dropped: <1e-6 on mean)
    pred = 1{l1 > l0}
    w    = 0.1 if pred==y else (1.0 if y==0 else 5.0)
    out  = per-task means of w*base + weighted total.

Device strategy (pure data-parallel over 8 cores):
  The label enters only through (a) the sign of u = (2y-1)*d and (b) the
  per-class weights. Both are handled WITHOUT shipping labels to the device:
  the host partitions each (core, task) shard's elements by label into two
  fixed-width column blocks (a pure label-derived permutation; the sum is
  permutation-invariant). Within a block the sign is a compile-time constant
  folded into the ACT Exp scale, and the weights collapse to a host-side
  linear combination of two block sums:
      S_B = sum(base)        -- free via Ln's accum_out
      S_Q = sum(q * base)    -- q = 1{d > 0}; reduced by TensorE ones-matmul
  y=1 block: sum(w*base)/2 = 2.5 *S_B - 2.45*S_Q   (w/2 = 2.5 - 2.45q; d=0 tie -> 5: exact)
  y=0 block: sum(w*base)/2 = 0.05*S_B + 0.45*S_Q   (w/2 = 0.05 + 0.45q; tie -> 0.1: exact)
  Blocks are padded to a fixed 128x4160 with inert elements (|d|=60 with the
  sign making exp underflow -> base ~ 1e-26, contributes nothing).

Per (task, block) tile [128, 4160] bf16:
  DVE:  d = l1 - l0;  q = is_gt(d, 0);  qb = q * base      (3 ops)
  ACT:  e = Exp(scale*d);  base = Ln(e + 1) with accum_out  (one shared table set)
  PE :  ones-matmul of qb into a per-(task,block) PSUM [1,512]
Host: bf16 deinterleave/partition prep, final f64 reduction + task weights.
"""

import os

import numpy as np
import ml_dtypes

import concourse.bacc as bacc
import concourse.mybir as mybir
from concourse import tile
from concourse.bass_utils import run_bass_kernel_spmd
from concourse.hw_specs import get_activation_tables

B = 8388608
N_CORES = 8
P = 128
SHARD = B // N_CORES          # 1048576 elements per core per task
C1 = 4160                     # padded columns per label block (max count 532480 >> 17 sigma)
TASKS = 3
NBLK = 2                      # block 0: y=1, block 1: y=0
MM = 512                      # matmul slice (one PSUM bank row)

BF16 = mybir.dt.bfloat16
F32 = mybir.dt.float32
AF = mybir.ActivationFunctionType
OP = mybir.AluOpType

# (exp scale, host coef on S_B, host coef on S_Q) per block
BLOCKS = [(-1.0, 2.5, -2.45), (1.0, 0.05, 0.45)]
PAD_D = 60.0  # pad element |d|; sign per block makes exp underflow

# exposed for test.py (harness ignores)
LAST_RESULTS = None


class _Bacc(bacc.Bacc):
    """Bacc that pins Exp and Ln to the shared natural_log_exp_and_others
    activation-table set (default placement alternates sets, paying a
    ~1.3us ACT_TABLE_LOAD per switch)."""

    def insert_act_table_loads(self):
        has_activation = any(
            isinstance(i, mybir.InstActivation)
            for b in self.main_func.blocks
            for i in b.instructions
        )
        if not has_activation:
            return
        combined = "natural_log_exp_and_others"
        tables = []
        for name, funcs in get_activation_tables(self.m.arch).items():
            if name != combined:
                funcs = funcs - {AF.Exp, AF.Ln}
            tables.append((name, funcs))
        bacc._bass_rust.insert_act_table_loads(self, tables)


def _build_nc():
    nc = _Bacc("TRN2")

    ins = {}
    for t in range(TASKS):
        for nm in ("l0", "l1"):
            ins[(t, nm)] = nc.dram_tensor(
                f"{nm}_{t}", [P, NBLK * C1], BF16, kind="ExternalInput"
            )
    out_qb = nc.dram_tensor("qb_out", [TASKS * NBLK, MM], F32, kind="ExternalOutput")
    out_b = nc.dram_tensor("b_out", [P, TASKS * NBLK * 2], F32, kind="ExternalOutput")

    with tile.TileContext(nc) as tc:
        with (
            tc.tile_pool(name="io", bufs=3) as io,
            tc.tile_pool(name="mid", bufs=2) as mid,
            tc.tile_pool(name="cst", bufs=1) as cst,
            tc.tile_pool(name="psum", bufs=1, space="PSUM") as psump,
        ):
            ones = cst.tile([P, 1], BF16)
            nc.vector.memset(ones[:], 1.0)
            accb = cst.tile([P, TASKS * NBLK * 2], F32)

            psums = []
            for i in range(TASKS * NBLK):
                psums.append(psump.tile([1, MM], F32, tag=f"ps{i}", name=f"ps{i}"))

            # Each (task, block) is split into a small lead-in tile plus a
            # large tile: the small one gets the ACT pipeline started while
            # the big DMAs are still in flight. accum_out is per-instruction,
            # so each sub-tile writes its own accb column.
            SPLITS = [(0, 1040), (1040, C1)]
            for t in range(TASKS):
                for g in range(NBLK):
                    idx = t * NBLK + g
                    scale, _, _ = BLOCKS[g]
                    first_mm = True
                    for si, (c_lo, c_hi) in enumerate(SPLITS):
                        cw = c_hi - c_lo
                        aidx = idx * len(SPLITS) + si
                        sl = slice(g * C1 + c_lo, g * C1 + c_hi)
                        l0 = io.tile([P, cw], BF16, tag=f"l0_{si}")
                        l1 = io.tile([P, cw], BF16, tag=f"l1_{si}")
                        nc.sync.dma_start(out=l0[:], in_=ins[(t, "l0")][:, sl])
                        nc.sync.dma_start(out=l1[:], in_=ins[(t, "l1")][:, sl])

                        d = mid.tile([P, cw], BF16, tag=f"d_{si}")
                        e = mid.tile([P, cw], BF16, tag=f"e_{si}")
                        base = mid.tile([P, cw], BF16, tag=f"base_{si}")
                        q = mid.tile([P, cw], BF16, tag=f"q_{si}")
                        qb = mid.tile([P, cw], BF16, tag=f"qb_{si}")

                        nc.vector.tensor_sub(out=d[:], in0=l1[:], in1=l0[:])
                        nc.scalar.activation(e[:], d[:], AF.Exp, bias=0.0, scale=scale)
                        nc.scalar.activation(
                            base[:], e[:], AF.Ln, bias=1.0, scale=1.0,
                            accum_out=accb[:, aidx : aidx + 1],
                        )
                        nc.vector.tensor_scalar(q[:], d[:], 0.0, None, OP.is_gt)
                        nc.vector.tensor_mul(out=qb[:], in0=q[:], in1=base[:])

                        nmm = (cw + MM - 1) // MM
                        for k in range(nmm):
                            lo = k * MM
                            hi = min(lo + MM, cw)
                            nc.tensor.matmul(
                                psums[idx][:, 0 : hi - lo],
                                ones[:],
                                qb[:, lo:hi],
                                start=first_mm,
                                stop=(si == len(SPLITS) - 1 and k == nmm - 1),
                            )
                            first_mm = False

            for i in range(TASKS * NBLK):
                qb_sb = cst.tile([1, MM], F32, tag=f"qbs{i}", name=f"qbs{i}")
                nc.vector.tensor_copy(out=qb_sb[:], in_=psums[i][:])
                nc.sync.dma_start(out=out_qb[i : i + 1, :], in_=qb_sb[:])
            nc.sync.dma_start(out=out_b[:, :], in_=accb[:])

    # Bacc defers register allocation to finalize(); the axon PJRT path
    # serializes the BIR without finalizing, so do it here.
    if not nc.is_finalized():
        nc.finalize()
    return nc


_NC_CACHE = None


def _get_nc():
    global _NC_CACHE
    if _NC_CACHE is None:
        _NC_CACHE = _build_nc()
    return _NC_CACHE


def _prep_task(logits: np.ndarray, targets: np.ndarray):
    """Per core: split the shard by label into two padded [P, C1] blocks
    (bf16), concatenated to [P, 2*C1] per logit plane."""
    bf = ml_dtypes.bfloat16
    l0 = logits[:, 0].astype(bf)
    l1 = logits[:, 1].astype(bf)
    y = np.asarray(targets).astype(np.int8)

    l0_planes = np.empty((N_CORES, P, NBLK * C1), dtype=bf)
    l1_planes = np.empty((N_CORES, P, NBLK * C1), dtype=bf)
    cap = P * C1
    for c in range(N_CORES):
        sl = slice(c * SHARD, (c + 1) * SHARD)
        yc = y[sl]
        for g, want in ((0, 1), (1, 0)):
            m = yc == want
            n = int(m.sum())
            if n > cap:
                raise ValueError(f"label block overflow: {n} > {cap}")
            # pad d = l1-l0 to +PAD_D (y=1 block) / -PAD_D (y=0 block)
            pad0 = -PAD_D / 2 if want == 1 else PAD_D / 2
            blk0 = np.full(cap, pad0, dtype=bf)
            blk1 = np.full(cap, -pad0, dtype=bf)
            blk0[:n] = l0[sl][m]
            blk1[:n] = l1[sl][m]
            l0_planes[c, :, g * C1 : (g + 1) * C1] = blk0.reshape(P, C1)
            l1_planes[c, :, g * C1 : (g + 1) * C1] = blk1.reshape(P, C1)
    return l0_planes, l1_planes


def kernel(logits_a, logits_b, logits_c, targets_a, targets_b, targets_c) -> np.ndarray:
    global LAST_RESULTS
    nc = _get_nc()

    planes = [
        _prep_task(np.asarray(logits_a), np.asarray(targets_a)),
        _prep_task(np.asarray(logits_b), np.asarray(targets_b)),
        _prep_task(np.asarray(logits_c), np.asarray(targets_c)),
    ]

    in_maps = []
    for c in range(N_CORES):
        m = {}
        for t in range(TASKS):
            l0p, l1p = planes[t]
            m[f"l0_{t}"] = l0p[c]
            m[f"l1_{t}"] = l1p[c]
        in_maps.append(m)

    want_trace = bool(os.environ.get("BASS_TRACE"))
    if want_trace:
        try:  # tracing needs the axon NTFF hook module; degrade if absent
            import antenv.axon_hooks  # noqa: F401
        except ImportError:
            want_trace = False
            os.environ["BASS_NEVER_TRACE"] = "1"

    res = run_bass_kernel_spmd(
        nc,
        in_maps,
        list(range(N_CORES)),
        trace=want_trace,
    )
    LAST_RESULTS = res

    half_sums = np.zeros(TASKS, dtype=np.float64)
    for c in range(N_CORES):
        qb = np.asarray(res.results[c]["qb_out"], dtype=np.float64)  # [6, MM]
        bb = np.asarray(res.results[c]["b_out"], dtype=np.float64)   # [P, 6]
        for t in range(TASKS):
            for g in range(NBLK):
                idx = t * NBLK + g
                _, ca, cb = BLOCKS[g]
                sb = bb[:, 2 * idx].sum() + bb[:, 2 * idx + 1].sum()
                half_sums[t] += ca * sb + cb * qb[idx].sum()
    means = 2.0 * half_sums / B
    la, lb, lc = means
    total = 1.0 * la + 0.5 * lb + 2.0 * lc
    return np.array([la, lb, lc, total], dtype=np.float32)


# revision 14
# speedup vs baseline: 1.0231x; 1.0231x over previous
"""Trainium2 Bass kernel for nn_BusinessCostLoss (weighted binary CE loss).

Reference math (per task, per element, labels y in {0,1}):
    d    = l1 - l0
    base = -log(softmax(l)[y]) = log(1 + exp(-(2y-1)*d))  (eps=1e-8 dropped: <1e-6 on mean)
    pred = 1{l1 > l0}
    w    = 0.1 if pred==y else (1.0 if y==0 else 5.0)
    out  = per-task means of w*base + weighted total.

Device strategy (pure data-parallel over 8 cores):
  The label enters only through (a) the sign of u = (2y-1)*d and (b) the
  per-class weights. Both are handled WITHOUT shipping labels to the device:
  the host partitions each (core, task) shard's elements by label into two
  fixed-width column blocks (a pure label-derived permutation; the sum is
  permutation-invariant). Within a block the sign is a compile-time constant
  folded into the ACT Exp scale, and the weights collapse to a host-side
  linear combination of two block sums:
      S_B = sum(base)        -- free via Ln's accum_out
      S_Q = sum(q * base)    -- q = 1{d > 0}; reduced by TensorE ones-matmul
  y=1 block: sum(w*base)/2 = 2.5 *S_B - 2.45*S_Q   (w/2 = 2.5 - 2.45q; d=0 tie -> 5: exact)
  y=0 block: sum(w*base)/2 = 0.05*S_B + 0.45*S_Q   (w/2 = 0.05 + 0.45q; tie -> 0.1: exact)
  Blocks are padded to a fixed 128x4160 with inert elements (|d|=60 with the
  sign making exp underflow -> base ~ 1e-26, contributes nothing).

Per (task, block) tile [128, 4160] bf16:
  DVE:  d = l1 - l0;  q = is_gt(d, 0);  qb = q * base      (3 ops)
  ACT:  e = Exp(scale*d);  base = Ln(e + 1) with accum_out  (one shared table set)
  PE :  ones-matmul of qb into a per-(task,block) PSUM [1,512]
Host: bf16 deinterleave/partition prep, final f64 reduction + task weights.
"""

import os

import numpy as np
import ml_dtypes

import concourse.bacc as bacc
import concourse.mybir as mybir
from concourse import tile
from concourse.bass_utils import run_bass_kernel_spmd
from concourse.hw_specs import get_activation_tables

B = 8388608
N_CORES = 8
P = 128
SHARD = B // N_CORES          # 1048576 elements per core per task
C1 = 4160                     # padded columns per label block (max count 532480 >> 17 sigma)
TASKS = 3
NBLK = 2                      # block 0: y=1, block 1: y=0
MM = 512                      # matmul slice (one PSUM bank row)

BF16 = mybir.dt.bfloat16
F32 = mybir.dt.float32
AF = mybir.ActivationFunctionType
OP = mybir.AluOpType

# (exp scale, host coef on S_B, host coef on S_Q) per block
BLOCKS = [(-1.0, 2.5, -2.45), (1.0, 0.05, 0.45)]
PAD_D = 60.0  # pad element |d|; sign per block makes exp underflow

# exposed for test.py (harness ignores)
LAST_RESULTS = None


class _Bacc(bacc.Bacc):
    """Bacc that pins Exp and Ln to the shared natural_log_exp_and_others
    activation-table set (default placement alternates sets, paying a
    ~1.3us ACT_TABLE_LOAD per switch)."""

    def insert_act_table_loads(self):
        has_activation = any(
            isinstance(i, mybir.InstActivation)
            for b in self.main_func.blocks
            for i in b.instructions
        )
        if not has_activation:
            return
        combined = "natural_log_exp_and_others"
        tables = []
        for name, funcs in get_activation_tables(self.m.arch).items():
            if name != combined:
                funcs = funcs - {AF.Exp, AF.Ln}
            tables.append((name, funcs))
        bacc._bass_rust.insert_act_table_loads(self, tables)


def _build_nc():
    nc = _Bacc("TRN2")

    ins = {}
    for t in range(TASKS):
        for nm in ("l0", "l1"):
            ins[(t, nm)] = nc.dram_tensor(
                f"{nm}_{t}", [P, NBLK * C1], BF16, kind="ExternalInput"
            )
    out_qb = nc.dram_tensor("qb_out", [TASKS * NBLK, MM], F32, kind="ExternalOutput")
    out_b = nc.dram_tensor("b_out", [P, TASKS * NBLK * 3], F32, kind="ExternalOutput")

    with tile.TileContext(nc) as tc:
        with (
            tc.tile_pool(name="io", bufs=3) as io,
            tc.tile_pool(name="mid", bufs=3) as mid,
            tc.tile_pool(name="cst", bufs=1) as cst,
            tc.tile_pool(name="psum", bufs=1, space="PSUM") as psump,
        ):
            ones = cst.tile([P, 1], BF16)
            nc.vector.memset(ones[:], 1.0)
            accb = cst.tile([P, TASKS * NBLK * 3], F32)

            psums = []
            for i in range(TASKS * NBLK):
                psums.append(psump.tile([1, MM], F32, tag=f"ps{i}", name=f"ps{i}"))

            # Each (task, block) is split into a small lead-in tile plus a
            # large tile: the small one gets the ACT pipeline started while
            # the big DMAs are still in flight. accum_out is per-instruction,
            # so each sub-tile writes its own accb column.
            SPLITS = [(0, 1040), (1040, 3120), (3120, C1)]
            for t in range(TASKS):
                for g in range(NBLK):
                    idx = t * NBLK + g
                    scale, _, _ = BLOCKS[g]
                    first_mm = True
                    for si, (c_lo, c_hi) in enumerate(SPLITS):
                        cw = c_hi - c_lo
                        aidx = idx * len(SPLITS) + si
                        sl = slice(g * C1 + c_lo, g * C1 + c_hi)
                        l0 = io.tile([P, cw], BF16, tag=f"l0_{si}")
                        l1 = io.tile([P, cw], BF16, tag=f"l1_{si}")
                        nc.sync.dma_start(out=l0[:], in_=ins[(t, "l0")][:, sl])
                        nc.sync.dma_start(out=l1[:], in_=ins[(t, "l1")][:, sl])

                        d = mid.tile([P, cw], BF16, tag=f"d_{si}")
                        e = mid.tile([P, cw], BF16, tag=f"e_{si}")
                        base = mid.tile([P, cw], BF16, tag=f"base_{si}")
                        q = mid.tile([P, cw], BF16, tag=f"q_{si}")
                        qb = mid.tile([P, cw], BF16, tag=f"qb_{si}")

                        nc.vector.tensor_sub(out=d[:], in0=l1[:], in1=l0[:])
                        nc.scalar.activation(e[:], d[:], AF.Exp, bias=0.0, scale=scale)
                        nc.scalar.activation(
                            base[:], e[:], AF.Ln, bias=1.0, scale=1.0,
                            accum_out=accb[:, aidx : aidx + 1],
                        )
                        nc.vector.tensor_scalar(q[:], d[:], 0.0, None, OP.is_gt)
                        nc.vector.tensor_mul(out=qb[:], in0=q[:], in1=base[:])

                        nmm = (cw + MM - 1) // MM
                        for k in range(nmm):
                            lo = k * MM
                            hi = min(lo + MM, cw)
                            nc.tensor.matmul(
                                psums[idx][:, 0 : hi - lo],
                                ones[:],
                                qb[:, lo:hi],
                                start=first_mm,
                                stop=(si == len(SPLITS) - 1 and k == nmm - 1),
                            )
                            first_mm = False

            for i in range(TASKS * NBLK):
                qb_sb = cst.tile([1, MM], F32, tag=f"qbs{i}", name=f"qbs{i}")
                nc.vector.tensor_copy(out=qb_sb[:], in_=psums[i][:])
                nc.sync.dma_start(out=out_qb[i : i + 1, :], in_=qb_sb[:])
            nc.sync.dma_start(out=out_b[:, :], in_=accb[:])

    # Bacc defers register allocation to finalize(); the axon PJRT path
    # serializes the BIR without finalizing, so do it here.
    if not nc.is_finalized():
        nc.finalize()
    return nc


_NC_CACHE = None


def _get_nc():
    global _NC_CACHE
    if _NC_CACHE is None:
        _NC_CACHE = _build_nc()
    return _NC_CACHE


def _prep_task(logits: np.ndarray, targets: np.ndarray):
    """Per core: split the shard by label into two padded [P, C1] blocks
    (bf16), concatenated to [P, 2*C1] per logit plane."""
    bf = ml_dtypes.bfloat16
    l0 = logits[:, 0].astype(bf)
    l1 = logits[:, 1].astype(bf)
    y = np.asarray(targets).astype(np.int8)

    l0_planes = np.empty((N_CORES, P, NBLK * C1), dtype=bf)
    l1_planes = np.empty((N_CORES, P, NBLK * C1), dtype=bf)
    cap = P * C1
    for c in range(N_CORES):
        sl = slice(c * SHARD, (c + 1) * SHARD)
        yc = y[sl]
        for g, want in ((0, 1), (1, 0)):
            m = yc == want
            n = int(m.sum())
            if n > cap:
                raise ValueError(f"label block overflow: {n} > {cap}")
            # pad d = l1-l0 to +PAD_D (y=1 block) / -PAD_D (y=0 block)
            pad0 = -PAD_D / 2 if want == 1 else PAD_D / 2
            blk0 = np.full(cap, pad0, dtype=bf)
            blk1 = np.full(cap, -pad0, dtype=bf)
            blk0[:n] = l0[sl][m]
            blk1[:n] = l1[sl][m]
            l0_planes[c, :, g * C1 : (g + 1) * C1] = blk0.reshape(P, C1)
            l1_planes[c, :, g * C1 : (g + 1) * C1] = blk1.reshape(P, C1)
    return l0_planes, l1_planes


def kernel(logits_a, logits_b, logits_c, targets_a, targets_b, targets_c) -> np.ndarray:
    global LAST_RESULTS
    nc = _get_nc()

    planes = [
        _prep_task(np.asarray(logits_a), np.asarray(targets_a)),
        _prep_task(np.asarray(logits_b), np.asarray(targets_b)),
        _prep_task(np.asarray(logits_c), np.asarray(targets_c)),
    ]

    in_maps = []
    for c in range(N_CORES):
        m = {}
        for t in range(TASKS):
            l0p, l1p = planes[t]
            m[f"l0_{t}"] = l0p[c]
            m[f"l1_{t}"] = l1p[c]
        in_maps.append(m)

    want_trace = bool(os.environ.get("BASS_TRACE"))
    if want_trace:
        try:  # tracing needs the axon NTFF hook module; degrade if absent
            import antenv.axon_hooks  # noqa: F401
        except ImportError:
            want_trace = False
            os.environ["BASS_NEVER_TRACE"] = "1"

    res = run_bass_kernel_spmd(
        nc,
        in_maps,
        list(range(N_CORES)),
        trace=want_trace,
    )
    LAST_RESULTS = res

    half_sums = np.zeros(TASKS, dtype=np.float64)
    for c in range(N_CORES):
        qb = np.asarray(res.results[c]["qb_out"], dtype=np.float64)  # [6, MM]
        bb = np.asarray(res.results[c]["b_out"], dtype=np.float64)   # [P, 6]
        for t in range(TASKS):
            for g in range(NBLK):
                idx = t * NBLK + g
                _, ca, cb = BLOCKS[g]
                sb = bb[:, 3 * idx : 3 * idx + 3].sum()
                half_sums[t] += ca * sb + cb * qb[idx].sum()
    means = 2.0 * half_sums / B
    la, lb, lc = means
    total = 1.0 * la + 0.5 * lb + 2.0 * lc
    return np.array([la, lb, lc, total], dtype=np.float32)


# revision 15
# speedup vs baseline: 1.0362x; 1.0128x over previous
"""Trainium2 Bass kernel for nn_BusinessCostLoss (weighted binary CE loss).

Reference math (per task, per element, labels y in {0,1}):
    d    = l1 - l0
    base = -log(softmax(l)[y]) = log(1 + exp(-(2y-1)*d))  (eps=1e-8 dropped: <1e-6 on mean)
    pred = 1{l1 > l0}
    w    = 0.1 if pred==y else (1.0 if y==0 else 5.0)
    out  = per-task means of w*base + weighted total.

Device strategy (pure data-parallel over 8 cores):
  The label enters only through (a) the sign of u = (2y-1)*d and (b) the
  per-class weights. Both are handled WITHOUT shipping labels to the device:
  the host partitions each (core, task) shard's elements by label into two
  fixed-width column blocks (a pure label-derived permutation; the sum is
  permutation-invariant). Within a block the sign is a compile-time constant
  folded into the ACT Exp scale, and the weights collapse to a host-side
  linear combination of two block sums:
      S_B = sum(base)        -- free via Ln's accum_out
      S_Q = sum(q * base)    -- q = 1{d > 0}; reduced by TensorE ones-matmul
  y=1 block: sum(w*base)/2 = 2.5 *S_B - 2.45*S_Q   (w/2 = 2.5 - 2.45q; d=0 tie -> 5: exact)
  y=0 block: sum(w*base)/2 = 0.05*S_B + 0.45*S_Q   (w/2 = 0.05 + 0.45q; tie -> 0.1: exact)
  Blocks are padded to a fixed 128x4160 with inert elements (|d|=60 with the
  sign making exp underflow -> base ~ 1e-26, contributes nothing).

Per (task, block) tile [128, 4160] bf16:
  DVE:  d = l1 - l0;  q = is_gt(d, 0);  qb = q * base      (3 ops)
  ACT:  e = Exp(scale*d);  base = Ln(e + 1) with accum_out  (one shared table set)
  PE :  ones-matmul of qb into a per-(task,block) PSUM [1,512]
Host: bf16 deinterleave/partition prep, final f64 reduction + task weights.
"""

import os

import numpy as np
import ml_dtypes

import concourse.bacc as bacc
import concourse.mybir as mybir
from concourse import tile
from concourse.bass_utils import run_bass_kernel_spmd
from concourse.hw_specs import get_activation_tables

B = 8388608
N_CORES = 8
P = 128
SHARD = B // N_CORES          # 1048576 elements per core per task
C1 = 4160                     # padded columns per label block (max count 532480 >> 17 sigma)
TASKS = 3
NBLK = 2                      # block 0: y=1, block 1: y=0
MM = 512                      # matmul slice (one PSUM bank row)

BF16 = mybir.dt.bfloat16
F32 = mybir.dt.float32
AF = mybir.ActivationFunctionType
OP = mybir.AluOpType

# (exp scale, host coef on S_B, host coef on S_Q) per block
BLOCKS = [(-1.0, 2.5, -2.45), (1.0, 0.05, 0.45)]
PAD_D = 60.0  # pad element |d|; sign per block makes exp underflow
NSPLIT = 2

# exposed for test.py (harness ignores)
LAST_RESULTS = None


class _Bacc(bacc.Bacc):
    """Bacc that pins Exp and Ln to the shared natural_log_exp_and_others
    activation-table set (default placement alternates sets, paying a
    ~1.3us ACT_TABLE_LOAD per switch)."""

    def insert_act_table_loads(self):
        has_activation = any(
            isinstance(i, mybir.InstActivation)
            for b in self.main_func.blocks
            for i in b.instructions
        )
        if not has_activation:
            return
        combined = "natural_log_exp_and_others"
        tables = []
        for name, funcs in get_activation_tables(self.m.arch).items():
            if name != combined:
                funcs = funcs - {AF.Exp, AF.Ln}
            tables.append((name, funcs))
        bacc._bass_rust.insert_act_table_loads(self, tables)


def _build_nc():
    nc = _Bacc("TRN2")

    ins = {}
    for t in range(TASKS):
        for nm in ("l0", "l1"):
            ins[(t, nm)] = nc.dram_tensor(
                f"{nm}_{t}", [P, NBLK * C1], BF16, kind="ExternalInput"
            )
    out_qb = nc.dram_tensor("qb_out", [TASKS * NBLK, MM], F32, kind="ExternalOutput")
    out_b = nc.dram_tensor("b_out", [P, TASKS * NBLK * NSPLIT], F32, kind="ExternalOutput")

    with tile.TileContext(nc) as tc:
        with (
            tc.tile_pool(name="io", bufs=3) as io,
            tc.tile_pool(name="mid", bufs=3) as mid,
            tc.tile_pool(name="cst", bufs=1) as cst,
            tc.tile_pool(name="psum", bufs=1, space="PSUM") as psump,
        ):
            ones = cst.tile([P, 1], BF16)
            nc.vector.memset(ones[:], 1.0)
            accb = cst.tile([P, TASKS * NBLK * NSPLIT], F32)

            psums = []
            for i in range(TASKS * NBLK):
                psums.append(psump.tile([1, MM], F32, tag=f"ps{i}", name=f"ps{i}"))

            # Each (task, block) is split into a small lead-in tile plus a
            # large tile: the small one gets the ACT pipeline started while
            # the big DMAs are still in flight. accum_out is per-instruction,
            # so each sub-tile writes its own accb column.
            SPLITS = [(0, 2080), (2080, C1)]
            for t in range(TASKS):
                for g in range(NBLK):
                    idx = t * NBLK + g
                    scale, _, _ = BLOCKS[g]
                    first_mm = True
                    for si, (c_lo, c_hi) in enumerate(SPLITS):
                        cw = c_hi - c_lo
                        aidx = idx * len(SPLITS) + si
                        sl = slice(g * C1 + c_lo, g * C1 + c_hi)
                        l0 = io.tile([P, cw], BF16, tag=f"l0_{si}")
                        l1 = io.tile([P, cw], BF16, tag=f"l1_{si}")
                        nc.sync.dma_start(out=l0[:], in_=ins[(t, "l0")][:, sl])
                        nc.sync.dma_start(out=l1[:], in_=ins[(t, "l1")][:, sl])

                        d = mid.tile([P, cw], BF16, tag=f"d_{si}")
                        e = mid.tile([P, cw], BF16, tag=f"e_{si}")
                        base = mid.tile([P, cw], BF16, tag=f"base_{si}")
                        q = mid.tile([P, cw], BF16, tag=f"q_{si}")
                        qb = mid.tile([P, cw], BF16, tag=f"qb_{si}")

                        nc.vector.tensor_sub(out=d[:], in0=l1[:], in1=l0[:])
                        nc.scalar.activation(e[:], d[:], AF.Exp, bias=0.0, scale=scale)
                        nc.scalar.activation(
                            base[:], e[:], AF.Ln, bias=1.0, scale=1.0,
                            accum_out=accb[:, aidx : aidx + 1],
                        )
                        nc.vector.tensor_scalar(q[:], d[:], 0.0, None, OP.is_gt)
                        nc.vector.tensor_mul(out=qb[:], in0=q[:], in1=base[:])

                        nmm = (cw + MM - 1) // MM
                        for k in range(nmm):
                            lo = k * MM
                            hi = min(lo + MM, cw)
                            nc.tensor.matmul(
                                psums[idx][:, 0 : hi - lo],
                                ones[:],
                                qb[:, lo:hi],
                                start=first_mm,
                                stop=(si == len(SPLITS) - 1 and k == nmm - 1),
                            )
                            first_mm = False

            for i in range(TASKS * NBLK):
                qb_sb = cst.tile([1, MM], F32, tag=f"qbs{i}", name=f"qbs{i}")
                nc.vector.tensor_copy(out=qb_sb[:], in_=psums[i][:])
                nc.sync.dma_start(out=out_qb[i : i + 1, :], in_=qb_sb[:])
            nc.sync.dma_start(out=out_b[:, :], in_=accb[:])

    # Bacc defers register allocation to finalize(); the axon PJRT path
    # serializes the BIR without finalizing, so do it here.
    if not nc.is_finalized():
        nc.finalize()
    return nc


_NC_CACHE = None


def _get_nc():
    global _NC_CACHE
    if _NC_CACHE is None:
        _NC_CACHE = _build_nc()
    return _NC_CACHE


def _prep_task(logits: np.ndarray, targets: np.ndarray):
    """Per core: split the shard by label into two padded [P, C1] blocks
    (bf16), concatenated to [P, 2*C1] per logit plane."""
    bf = ml_dtypes.bfloat16
    l0 = logits[:, 0].astype(bf)
    l1 = logits[:, 1].astype(bf)
    y = np.asarray(targets).astype(np.int8)

    l0_planes = np.empty((N_CORES, P, NBLK * C1), dtype=bf)
    l1_planes = np.empty((N_CORES, P, NBLK * C1), dtype=bf)
    cap = P * C1
    for c in range(N_CORES):
        sl = slice(c * SHARD, (c + 1) * SHARD)
        yc = y[sl]
        for g, want in ((0, 1), (1, 0)):
            m = yc == want
            n = int(m.sum())
            if n > cap:
                raise ValueError(f"label block overflow: {n} > {cap}")
            # pad d = l1-l0 to +PAD_D (y=1 block) / -PAD_D (y=0 block)
            pad0 = -PAD_D / 2 if want == 1 else PAD_D / 2
            blk0 = np.full(cap, pad0, dtype=bf)
            blk1 = np.full(cap, -pad0, dtype=bf)
            blk0[:n] = l0[sl][m]
            blk1[:n] = l1[sl][m]
            l0_planes[c, :, g * C1 : (g + 1) * C1] = blk0.reshape(P, C1)
            l1_planes[c, :, g * C1 : (g + 1) * C1] = blk1.reshape(P, C1)
    return l0_planes, l1_planes


def kernel(logits_a, logits_b, logits_c, targets_a, targets_b, targets_c) -> np.ndarray:
    global LAST_RESULTS
    nc = _get_nc()

    planes = [
        _prep_task(np.asarray(logits_a), np.asarray(targets_a)),
        _prep_task(np.asarray(logits_b), np.asarray(targets_b)),
        _prep_task(np.asarray(logits_c), np.asarray(targets_c)),
    ]

    in_maps = []
    for c in range(N_CORES):
        m = {}
        for t in range(TASKS):
            l0p, l1p = planes[t]
            m[f"l0_{t}"] = l0p[c]
            m[f"l1_{t}"] = l1p[c]
        in_maps.append(m)

    want_trace = bool(os.environ.get("BASS_TRACE"))
    if want_trace:
        try:  # tracing needs the axon NTFF hook module; degrade if absent
            import antenv.axon_hooks  # noqa: F401
        except ImportError:
            want_trace = False
            os.environ["BASS_NEVER_TRACE"] = "1"

    res = run_bass_kernel_spmd(
        nc,
        in_maps,
        list(range(N_CORES)),
        trace=want_trace,
    )
    LAST_RESULTS = res

    half_sums = np.zeros(TASKS, dtype=np.float64)
    for c in range(N_CORES):
        qb = np.asarray(res.results[c]["qb_out"], dtype=np.float64)  # [6, MM]
        bb = np.asarray(res.results[c]["b_out"], dtype=np.float64)   # [P, 6]
        for t in range(TASKS):
            for g in range(NBLK):
                idx = t * NBLK + g
                _, ca, cb = BLOCKS[g]
                sb = bb[:, NSPLIT * idx : NSPLIT * idx + NSPLIT].sum()
                half_sums[t] += ca * sb + cb * qb[idx].sum()
    means = 2.0 * half_sums / B
    la, lb, lc = means
    total = 1.0 * la + 0.5 * lb + 2.0 * lc
    return np.array([la, lb, lc, total], dtype=np.float32)


# revision 16
# speedup vs baseline: 1.0384x; 1.0021x over previous
"""Trainium2 Bass kernel for nn_BusinessCostLoss (weighted binary CE loss).

Reference math (per task, per element, labels y in {0,1}):
    d    = l1 - l0
    base = -log(softmax(l)[y]) = log(1 + exp(-(2y-1)*d))  (eps=1e-8 dropped: <1e-6 on mean)
    pred = 1{l1 > l0}
    w    = 0.1 if pred==y else (1.0 if y==0 else 5.0)
    out  = per-task means of w*base + weighted total.

Device strategy (pure data-parallel over 8 cores):
  The label enters only through (a) the sign of u = (2y-1)*d and (b) the
  per-class weights. Both are handled WITHOUT shipping labels to the device:
  the host partitions each (core, task) shard's elements by label into two
  fixed-width column blocks (a pure label-derived permutation; the sum is
  permutation-invariant). Within a block the sign is a compile-time constant
  folded into the ACT Exp scale, and the weights collapse to a host-side
  linear combination of two block sums:
      S_B = sum(base)        -- free via Ln's accum_out
      S_Q = sum(q * base)    -- q = 1{d > 0}; reduced by TensorE ones-matmul
  y=1 block: sum(w*base)/2 = 2.5 *S_B - 2.45*S_Q   (w/2 = 2.5 - 2.45q; d=0 tie -> 5: exact)
  y=0 block: sum(w*base)/2 = 0.05*S_B + 0.45*S_Q   (w/2 = 0.05 + 0.45q; tie -> 0.1: exact)
  Blocks are padded to a fixed 128x4160 with inert elements (|d|=60 with the
  sign making exp underflow -> base ~ 1e-26, contributes nothing).

Per (task, block) tile [128, 4160] bf16:
  DVE:  d = l1 - l0;  q = is_gt(d, 0);  qb = q * base      (3 ops)
  ACT:  e = Exp(scale*d);  base = Ln(e + 1) with accum_out  (one shared table set)
  PE :  ones-matmul of qb into a per-(task,block) PSUM [1,512]
Host: bf16 deinterleave/partition prep, final f64 reduction + task weights.
"""

import os

import numpy as np
import ml_dtypes

import concourse.bacc as bacc
import concourse.mybir as mybir
from concourse import tile
from concourse.bass_utils import run_bass_kernel_spmd
from concourse.hw_specs import get_activation_tables

B = 8388608
N_CORES = 8
P = 128
SHARD = B // N_CORES          # 1048576 elements per core per task
C1 = 4160                     # padded columns per label block (max count 532480 >> 17 sigma)
TASKS = 3
NBLK = 2                      # block 0: y=1, block 1: y=0
MM = 512                      # matmul slice (one PSUM bank row)

BF16 = mybir.dt.bfloat16
F32 = mybir.dt.float32
AF = mybir.ActivationFunctionType
OP = mybir.AluOpType

# (exp scale, host coef on S_B, host coef on S_Q) per block
BLOCKS = [(-1.0, 2.5, -2.45), (1.0, 0.05, 0.45)]
PAD_D = 60.0  # pad element |d|; sign per block makes exp underflow
NSPLIT = 2

# exposed for test.py (harness ignores)
LAST_RESULTS = None


class _Bacc(bacc.Bacc):
    """Bacc that pins Exp and Ln to the shared natural_log_exp_and_others
    activation-table set (default placement alternates sets, paying a
    ~1.3us ACT_TABLE_LOAD per switch)."""

    def insert_act_table_loads(self):
        has_activation = any(
            isinstance(i, mybir.InstActivation)
            for b in self.main_func.blocks
            for i in b.instructions
        )
        if not has_activation:
            return
        combined = "natural_log_exp_and_others"
        tables = []
        for name, funcs in get_activation_tables(self.m.arch).items():
            if name != combined:
                funcs = funcs - {AF.Exp, AF.Ln}
            tables.append((name, funcs))
        bacc._bass_rust.insert_act_table_loads(self, tables)


def _build_nc():
    nc = _Bacc("TRN2")

    ins = {}
    for t in range(TASKS):
        for nm in ("l0", "l1"):
            ins[(t, nm)] = nc.dram_tensor(
                f"{nm}_{t}", [P, NBLK * C1], BF16, kind="ExternalInput"
            )
    out_qb = nc.dram_tensor("qb_out", [TASKS * NBLK, MM], F32, kind="ExternalOutput")
    out_b = nc.dram_tensor("b_out", [P, TASKS * NBLK * NSPLIT], F32, kind="ExternalOutput")

    with tile.TileContext(nc) as tc:
        with (
            tc.tile_pool(name="io", bufs=4) as io,
            tc.tile_pool(name="mid", bufs=3) as mid,
            tc.tile_pool(name="cst", bufs=1) as cst,
            tc.tile_pool(name="psum", bufs=1, space="PSUM") as psump,
        ):
            ones = cst.tile([P, 1], BF16)
            nc.vector.memset(ones[:], 1.0)
            accb = cst.tile([P, TASKS * NBLK * NSPLIT], F32)

            psums = []
            for i in range(TASKS * NBLK):
                psums.append(psump.tile([1, MM], F32, tag=f"ps{i}", name=f"ps{i}"))

            # Each (task, block) is split into a small lead-in tile plus a
            # large tile: the small one gets the ACT pipeline started while
            # the big DMAs are still in flight. accum_out is per-instruction,
            # so each sub-tile writes its own accb column.
            SPLITS = [(0, 2080), (2080, C1)]
            for t in range(TASKS):
                for g in range(NBLK):
                    idx = t * NBLK + g
                    scale, _, _ = BLOCKS[g]
                    first_mm = True
                    for si, (c_lo, c_hi) in enumerate(SPLITS):
                        cw = c_hi - c_lo
                        aidx = idx * len(SPLITS) + si
                        sl = slice(g * C1 + c_lo, g * C1 + c_hi)
                        l0 = io.tile([P, cw], BF16, tag=f"l0_{si}")
                        l1 = io.tile([P, cw], BF16, tag=f"l1_{si}")
                        nc.sync.dma_start(out=l0[:], in_=ins[(t, "l0")][:, sl])
                        nc.sync.dma_start(out=l1[:], in_=ins[(t, "l1")][:, sl])

                        d = mid.tile([P, cw], BF16, tag=f"d_{si}")
                        e = mid.tile([P, cw], BF16, tag=f"e_{si}")
                        base = mid.tile([P, cw], BF16, tag=f"base_{si}")
                        q = mid.tile([P, cw], BF16, tag=f"q_{si}")
                        qb = mid.tile([P, cw], BF16, tag=f"qb_{si}")

                        nc.vector.tensor_sub(out=d[:], in0=l1[:], in1=l0[:])
                        nc.scalar.activation(e[:], d[:], AF.Exp, bias=0.0, scale=scale)
                        nc.scalar.activation(
                            base[:], e[:], AF.Ln, bias=1.0, scale=1.0,
                            accum_out=accb[:, aidx : aidx + 1],
                        )
                        nc.vector.tensor_scalar(q[:], d[:], 0.0, None, OP.is_gt)
                        nc.vector.tensor_mul(out=qb[:], in0=q[:], in1=base[:])

                        nmm = (cw + MM - 1) // MM
                        for k in range(nmm):
                            lo = k * MM
                            hi = min(lo + MM, cw)
                            nc.tensor.matmul(
                                psums[idx][:, 0 : hi - lo],
                                ones[:],
                                qb[:, lo:hi],
                                start=first_mm,
                                stop=(si == len(SPLITS) - 1 and k == nmm - 1),
                            )
                            first_mm = False

            for i in range(TASKS * NBLK):
                qb_sb = cst.tile([1, MM], F32, tag=f"qbs{i}", name=f"qbs{i}")
                nc.vector.tensor_copy(out=qb_sb[:], in_=psums[i][:])
                nc.sync.dma_start(out=out_qb[i : i + 1, :], in_=qb_sb[:])
            nc.sync.dma_start(out=out_b[:, :], in_=accb[:])

    # Bacc defers register allocation to finalize(); the axon PJRT path
    # serializes the BIR without finalizing, so do it here.
    if not nc.is_finalized():
        nc.finalize()
    return nc


_NC_CACHE = None


def _get_nc():
    global _NC_CACHE
    if _NC_CACHE is None:
        _NC_CACHE = _build_nc()
    return _NC_CACHE


def _prep_task(logits: np.ndarray, targets: np.ndarray):
    """Per core: split the shard by label into two padded [P, C1] blocks
    (bf16), concatenated to [P, 2*C1] per logit plane."""
    bf = ml_dtypes.bfloat16
    l0 = logits[:, 0].astype(bf)
    l1 = logits[:, 1].astype(bf)
    y = np.asarray(targets).astype(np.int8)

    l0_planes = np.empty((N_CORES, P, NBLK * C1), dtype=bf)
    l1_planes = np.empty((N_CORES, P, NBLK * C1), dtype=bf)
    cap = P * C1
    for c in range(N_CORES):
        sl = slice(c * SHARD, (c + 1) * SHARD)
        yc = y[sl]
        for g, want in ((0, 1), (1, 0)):
            m = yc == want
            n = int(m.sum())
            if n > cap:
                raise ValueError(f"label block overflow: {n} > {cap}")
            # pad d = l1-l0 to +PAD_D (y=1 block) / -PAD_D (y=0 block)
            pad0 = -PAD_D / 2 if want == 1 else PAD_D / 2
            blk0 = np.full(cap, pad0, dtype=bf)
            blk1 = np.full(cap, -pad0, dtype=bf)
            blk0[:n] = l0[sl][m]
            blk1[:n] = l1[sl][m]
            l0_planes[c, :, g * C1 : (g + 1) * C1] = blk0.reshape(P, C1)
            l1_planes[c, :, g * C1 : (g + 1) * C1] = blk1.reshape(P, C1)
    return l0_planes, l1_planes


def kernel(logits_a, logits_b, logits_c, targets_a, targets_b, targets_c) -> np.ndarray:
    global LAST_RESULTS
    nc = _get_nc()

    planes = [
        _prep_task(np.asarray(logits_a), np.asarray(targets_a)),
        _prep_task(np.asarray(logits_b), np.asarray(targets_b)),
        _prep_task(np.asarray(logits_c), np.asarray(targets_c)),
    ]

    in_maps = []
    for c in range(N_CORES):
        m = {}
        for t in range(TASKS):
            l0p, l1p = planes[t]
            m[f"l0_{t}"] = l0p[c]
            m[f"l1_{t}"] = l1p[c]
        in_maps.append(m)

    want_trace = bool(os.environ.get("BASS_TRACE"))
    if want_trace:
        try:  # tracing needs the axon NTFF hook module; degrade if absent
            import antenv.axon_hooks  # noqa: F401
        except ImportError:
            want_trace = False
            os.environ["BASS_NEVER_TRACE"] = "1"

    res = run_bass_kernel_spmd(
        nc,
        in_maps,
        list(range(N_CORES)),
        trace=want_trace,
    )
    LAST_RESULTS = res

    half_sums = np.zeros(TASKS, dtype=np.float64)
    for c in range(N_CORES):
        qb = np.asarray(res.results[c]["qb_out"], dtype=np.float64)  # [6, MM]
        bb = np.asarray(res.results[c]["b_out"], dtype=np.float64)   # [P, 6]
        for t in range(TASKS):
            for g in range(NBLK):
                idx = t * NBLK + g
                _, ca, cb = BLOCKS[g]
                sb = bb[:, NSPLIT * idx : NSPLIT * idx + NSPLIT].sum()
                half_sums[t] += ca * sb + cb * qb[idx].sum()
    means = 2.0 * half_sums / B
    la, lb, lc = means
    total = 1.0 * la + 0.5 * lb + 2.0 * lc
    return np.array([la, lb, lc, total], dtype=np.float32)
